# revision 2
# baseline (speedup 1.0000x reference)
"""KL(N(prior_mu, diag(prior_sigma^2)) || N(post_mu, diag(post_sigma^2))) mean loss.

Data-parallel over batch dim B=32 across 8 NeuronCores (4 batches/core).
Host casts prior_sigma to bf16 and post_sigma/prior_mu/post_mu to fp8-e3m4
(5 B/elem -> 40 KiB/partition/core); the 2e-2 rel-err budget absorbs the
~0.1% quantization bias.

Math per element (sp, sq, mp, mq), with m = sp/sq and v = (mq-mp)/sq:
  kl = 0.5*m^2 + 0.5*v^2 - 0.5 - ln m
  Sum kl = 0.5*(Sum m^2 + Sum v^2) - N/2 - Sum ln m

Engine split per tile [128, w]:
  ACT : rcp = Reciprocal(sq) (PWP reciprocal table; the bass wrapper blocks
        Af.Reciprocal so the InstActivation is emitted directly -- one ACT
        pass instead of the Ln+Exp pair), the sq-stream DMAs, and the final
        root-Ln passes (+1 table switch).
  DVE : m = sp*rcp, v = d*rcp (bf16 2x rate); tree levels l2, l3; psum
        drains at the end.
  Pool: d = mq - mp (fp8 in, bf16 out); tree level l1.
  PE  : Sum m^2 and Sum v^2 as gram matrices G += X_chunk^T X_chunk
        accumulated over all [128,128] chunks in two psum banks; the host
        reads the traces. This removes every square/STT pass from DVE.
  SP  : sp- and mm-stream DMAs, output DMAs.

Sum ln m comes from a bf16 pairwise product tree over m (groups of 8;
m in [1/3, 3] so group products stay in bf16 range) and ACT Ln+accum over
the [128, 1024] tree roots in two chunks so most of it overlaps the last
tile. DMA is split across the SP and ACT HWDGE queues, which transfer
concurrently.

Raw Bass (no Tile): standalone wait_ge synchronization with a schedule
prepass assigning per-engine ordinals, per-slot DMA semaphores, 3 DMA
slots, parity (2-slot) intermediate buffers.
"""

import sys
from contextlib import ExitStack

sys.path.insert(0, "/opt/trn_rl_repo")

import numpy as np
import ml_dtypes

import concourse.bass as bass
from concourse import mybir
from concourse.bass_utils import run_bass_kernel_spmd

B, L, N, D = 32, 128, 32, 64
NCORES = 8
BPC = B // NCORES
ELEMS = BPC * L * N * D          # 1_048_576 per tensor per core
P = 128
WIDTHS = [2048, 2048, 2048, 1536, 512]
NT = len(WIDTHS)
assert sum(WIDTHS) * P == ELEMS
NSLOT = 3
GRP = 8                           # product-tree group size (3 levels)
SROOT = sum(w // GRP for w in WIDTHS)     # 1024 tree-root columns
WMAX = max(WIDTHS)

_CACHE = {}


def _build():
    dt = mybir.dt
    Af = mybir.ActivationFunctionType
    Op = mybir.AluOpType

    nc = bass.Bass()
    sq8 = nc.declare_dram_parameter("sq8", [ELEMS], dt.float8e3, isOutput=False)
    spb = nc.declare_dram_parameter("spb", [ELEMS], dt.bfloat16, isOutput=False)
    mm8 = nc.declare_dram_parameter("mm8", [2 * ELEMS], dt.float8e3,
                                    isOutput=False)
    # stats: col 0,1 = root-Ln accums (Sum ln m in two chunks)
    stats = nc.declare_dram_parameter("stats", [P, 2], dt.float32, isOutput=True)
    # gram: [:, 0:128] = G_m, [:, 128:256] = G_v; host takes the traces
    gram = nc.declare_dram_parameter("gram", [P, 2 * P], dt.float32,
                                     isOutput=True)

    off1 = [0]
    off2 = [0]
    for w in WIDTHS:
        off1.append(off1[-1] + P * w)
        off2.append(off2[-1] + P * 2 * w)

    def dram1(t, i):
        return t[off1[i]: off1[i + 1]].rearrange("(p f) -> p f", p=P)

    def dram2(t, i):
        return t[off2[i]: off2[i + 1]].rearrange("(p f) -> p f", p=P)

    soff = [0]
    for w in WIDTHS:
        soff.append(soff[-1] + w // GRP)

    # --- ordinal prepass (mirrors emission order exactly) ---
    # ACT sa: dummy_rcp=1; rcp_k=k+2; dummy_ln=NT+2; ln1=NT+3; ln2=NT+4
    rcpod = [k + 2 for k in range(NT)]
    ln1od = NT + 3
    ln2od = NT + 4
    # DVE sv: m_k=2k+1, v_k=2k+2; dGm=2NT+1, dGv=2NT+2
    mod_ = [2 * k + 1 for k in range(NT)]
    vod = [2 * k + 2 for k in range(NT)]
    dgvod = 2 * NT + 2
    # Pool sg: d_k=4k+1, l1_k=4k+2, l2_k=4k+3, l3_k=4k+4
    dod = [4 * k + 1 for k in range(NT)]
    l1od = [4 * k + 2 for k in range(NT)]
    l2od = [4 * k + 3 for k in range(NT)]
    l3od = [4 * k + 4 for k in range(NT)]
    # PE se: gm_k=2k+1 (after last m-chunk), gv_k=2k+2
    gmod = [2 * k + 1 for k in range(NT)]
    gvod = [2 * k + 2 for k in range(NT)]
    pe_last = 2 * NT

    def ndma(i):
        return 16 * (i // NSLOT + 1)

    def raw_act(out, in_, func, scale=1.0, accum_out=None):
        eng = nc.scalar
        ins = [eng.lower_ap(in_),
               mybir.ImmediateValue(dtype=dt.float32, value=0.0),
               mybir.ImmediateValue(dtype=dt.float32, value=scale),
               mybir.ImmediateValue(dtype=dt.float32, value=0.0)]
        outs = [eng.lower_ap(out)]
        if accum_out is not None:
            outs.append(eng.lower_ap(accum_out))
        return eng.add_instruction(mybir.InstActivation(
            name=nc.get_next_instruction_name(), func=func, ins=ins, outs=outs))

    with ExitStack() as ctx:
        en = ctx.enter_context
        sq_b = [en(nc.sbuf_tensor(f"sq{i}", [P, WMAX], dt.float8e3))
                for i in range(NSLOT)]
        sp_b = [en(nc.sbuf_tensor(f"sp{i}", [P, WMAX], dt.bfloat16))
                for i in range(NSLOT)]
        mm_b = [en(nc.sbuf_tensor(f"mm{i}", [P, 2 * WMAX], dt.float8e3))
                for i in range(NSLOT)]
        rcp_b = [en(nc.sbuf_tensor(f"rcp{i}", [P, WMAX], dt.bfloat16))
                 for i in range(2)]
        m_b = [en(nc.sbuf_tensor(f"m{i}", [P, WMAX], dt.bfloat16))
               for i in range(2)]
        v_b = [en(nc.sbuf_tensor(f"v{i}", [P, WMAX], dt.bfloat16))
               for i in range(2)]
        d_b = [en(nc.sbuf_tensor(f"d{i}", [P, WMAX], dt.bfloat16))
               for i in range(2)]
        l1_b = [en(nc.sbuf_tensor(f"l1{i}", [P, WMAX // 2], dt.bfloat16))
                for i in range(2)]
        l2_b = [en(nc.sbuf_tensor(f"l2{i}", [P, WMAX // 4], dt.bfloat16))
                for i in range(2)]
        stash = en(nc.sbuf_tensor("stash", [P, SROOT], dt.bfloat16))
        lnout = en(nc.sbuf_tensor("lnout", [P, SROOT], dt.bfloat16))
        scr = en(nc.sbuf_tensor("scr", [P, 2], dt.bfloat16))
        st = en(nc.sbuf_tensor("st", [P, 2], dt.float32))
        gsb = en(nc.sbuf_tensor("gsb", [P, 2 * P], dt.float32))
        Gm = en(nc.psum_tensor("Gm", [P, P], dt.float32))
        Gv = en(nc.psum_tensor("Gv", [P, P], dt.float32))

        dsq = [en(nc.semaphore(f"dsq{i}")) for i in range(NSLOT)]
        dsp = [en(nc.semaphore(f"dsp{i}")) for i in range(NSLOT)]
        dmm = [en(nc.semaphore(f"dmm{i}")) for i in range(NSLOT)]
        sa = en(nc.semaphore("sa"))
        sv = en(nc.semaphore("sv"))
        sg = en(nc.semaphore("sg"))
        se = en(nc.semaphore("se"))
        do = en(nc.semaphore("do"))

        block = en(nc.Block())

        @block.sync
        def _(sync):
            for k in range(NT):
                s, w = k % NSLOT, WIDTHS[k]
                if k >= NSLOT:
                    sync.wait_ge(sv, mod_[k - NSLOT])   # sp slot WAR
                sync.dma_start(sp_b[s][:, 0:w], dram1(spb, k)).then_inc(dsp[s], 16)
                if k >= NSLOT:
                    sync.wait_ge(sg, dod[k - NSLOT])    # mm slot WAR
                sync.dma_start(mm_b[s][:, 0:2 * w], dram2(mm8, k)).then_inc(dmm[s], 16)
            sync.wait_ge(sa, ln2od)
            sync.wait_ge(sv, dgvod)
            sync.dma_start(stats[:, :], st[:, :]).then_inc(do, 16)
            sync.dma_start(gram[:, :], gsb[:, :]).then_inc(do, 16)
            sync.wait_ge(do, 32)

        @block.scalar
        def _(scalar):
            ones = nc.const_aps.tensor(1.0, (P, 1), dt.float32)
            # reciprocal table preload under the first DMAs
            raw_act(scr[:, 0:1], ones, Af.Reciprocal).then_inc(sa, 1)
            for k in range(min(NSLOT, NT)):
                s, w = k % NSLOT, WIDTHS[k]
                scalar.dma_start(sq_b[s][:, 0:w], dram1(sq8, k)).then_inc(dsq[s], 16)
            for k in range(NT):
                s, w = k % NSLOT, WIDTHS[k]
                scalar.wait_ge(dsq[s], ndma(k))
                if k >= 2:
                    scalar.wait_ge(sv, vod[k - 2])      # rcp WAR vs DVE v
                raw_act(rcp_b[k % 2][:, 0:w], sq_b[s][:, 0:w],
                        Af.Reciprocal).then_inc(sa, 1)
                kn = k + NSLOT
                if kn < NT:
                    sn, wn = kn % NSLOT, WIDTHS[kn]
                    scalar.wait_ge(sa, rcpod[k])        # sq slot WAR (self)
                    scalar.dma_start(sq_b[sn][:, 0:wn],
                                     dram1(sq8, kn)).then_inc(dsq[sn], 16)
            # switch to the natural_log table while Pool finishes the tree
            nc.scalar.activation(scr[:, 1:2], ones, Af.Ln).then_inc(sa, 1)
            # root-Ln part 1: tiles 0..NT-2
            scalar.wait_ge(sg, l3od[NT - 2])
            nc.scalar.activation(
                lnout[:, 0: soff[NT - 1]], stash[:, 0: soff[NT - 1]], Af.Ln,
                accum_out=st[:, 0:1]).then_inc(sa, 1)
            # part 2: last tile's roots
            scalar.wait_ge(sg, l3od[NT - 1])
            nc.scalar.activation(
                lnout[:, soff[NT - 1]: SROOT], stash[:, soff[NT - 1]: SROOT],
                Af.Ln, accum_out=st[:, 1:2]).then_inc(sa, 1)

        @block.vector
        def _(vector):
            for k in range(NT):
                s, j, w = k % NSLOT, k % 2, WIDTHS[k]
                vector.wait_ge(sa, rcpod[k])            # rcp RAW
                vector.wait_ge(dsp[s], ndma(k))
                if k >= 2:
                    vector.wait_ge(sg, l1od[k - 2])     # m WAR vs Pool l1
                    vector.wait_ge(se, gmod[k - 2])     # m WAR vs PE
                nc.vector.tensor_tensor(
                    m_b[j][:, 0:w], sp_b[s][:, 0:w], rcp_b[j][:, 0:w],
                    op=Op.mult).then_inc(sv, 1)
                vector.wait_ge(sg, dod[k])              # d RAW
                if k >= 2:
                    vector.wait_ge(se, gvod[k - 2])     # v WAR vs PE
                nc.vector.tensor_tensor(
                    v_b[j][:, 0:w], d_b[j][:, 0:w], rcp_b[j][:, 0:w],
                    op=Op.mult).then_inc(sv, 1)
            # psum drains
            vector.wait_ge(se, pe_last)
            nc.vector.tensor_copy(gsb[:, 0:P], Gm[:, :]).then_inc(sv, 1)
            nc.vector.tensor_copy(gsb[:, P:2 * P], Gv[:, :]).then_inc(sv, 1)

        @block.gpsimd
        def _(gpsimd):
            for k in range(NT):
                s, j, w = k % NSLOT, k % 2, WIDTHS[k]
                gpsimd.wait_ge(dmm[s], ndma(k))
                if k >= 2:
                    gpsimd.wait_ge(sv, vod[k - 2])      # d WAR vs DVE v
                nc.gpsimd.tensor_tensor(
                    d_b[j][:, 0:w], mm_b[s][:, w:2 * w], mm_b[s][:, 0:w],
                    op=Op.subtract).then_inc(sg, 1)
                gpsimd.wait_ge(sv, mod_[k])             # m RAW
                if k >= 2:
                    gpsimd.wait_ge(sg, l2od[k - 2])     # l1 WAR (self)
                nc.gpsimd.tensor_tensor(
                    l1_b[j][:, 0:w // 2], m_b[j][:, 0:w // 2],
                    m_b[j][:, w // 2:w], op=Op.mult).then_inc(sg, 1)
                gpsimd.wait_ge(sg, l1od[k])             # l1 RAW (self)
                if k >= 2:
                    gpsimd.wait_ge(sg, l3od[k - 2])     # l2 WAR (self)
                nc.gpsimd.tensor_tensor(
                    l2_b[j][:, 0:w // 4], l1_b[j][:, 0:w // 4],
                    l1_b[j][:, w // 4:w // 2], op=Op.mult).then_inc(sg, 1)
                gpsimd.wait_ge(sg, l2od[k])             # l2 RAW (self)
                nc.gpsimd.tensor_tensor(
                    stash[:, soff[k]: soff[k + 1]], l2_b[j][:, 0:w // 8],
                    l2_b[j][:, w // 8:w // 4], op=Op.mult).then_inc(sg, 1)

        @block.tensor
        def _(tensor):
            for k in range(NT):
                j, w = k % 2, WIDTHS[k]
                nch = w // P
                tensor.wait_ge(sv, mod_[k])
                for c in range(nch):
                    mm_ = nc.tensor.matmul(
                        Gm[:, :], m_b[j][:, c * P:(c + 1) * P],
                        m_b[j][:, c * P:(c + 1) * P],
                        start=(k == 0 and c == 0),
                        stop=(k == NT - 1 and c == nch - 1),
                        skip_group_check=True)
                    if c == nch - 1:
                        mm_.then_inc(se, 1)
                tensor.wait_ge(sv, vod[k])
                for c in range(nch):
                    mm_ = nc.tensor.matmul(
                        Gv[:, :], v_b[j][:, c * P:(c + 1) * P],
                        v_b[j][:, c * P:(c + 1) * P],
                        start=(k == 0 and c == 0),
                        stop=(k == NT - 1 and c == nch - 1),
                        skip_group_check=True)
                    if c == nch - 1:
                        mm_.then_inc(se, 1)

    return nc


def _get_nc():
    if "nc" not in _CACHE:
        _CACHE["nc"] = _build()
    return _CACHE["nc"]


def _pack(inputs):
    """Per-core packed streams, tile-blocked to match the kernel's DRAM APs:
    sq8 = post_sigma fp8-e3m4; spb = prior_sigma bf16;
    mm8 = per tile [prior_mu | post_mu] fp8-e3m4 (so d = cols w:2w - 0:w)."""
    e3 = ml_dtypes.float8_e3m4
    bf = ml_dtypes.bfloat16
    in_maps = []
    for k in range(NCORES):
        sl = slice(k * BPC, (k + 1) * BPC)
        sq = np.ascontiguousarray(inputs["post_sigma"][sl]).reshape(-1).astype(e3)
        sp = np.ascontiguousarray(inputs["prior_sigma"][sl]).reshape(-1).astype(bf)
        mp = np.ascontiguousarray(inputs["prior_mu"][sl]).reshape(-1).astype(e3)
        mq = np.ascontiguousarray(inputs["post_mu"][sl]).reshape(-1).astype(e3)
        mm_blocks = []
        pos = 0
        for w in WIDTHS:
            n = P * w
            a = mp[pos:pos + n].reshape(P, w)
            b = mq[pos:pos + n].reshape(P, w)
            mm_blocks.append(np.concatenate([a, b], axis=1).ravel())
            pos += n
        in_maps.append({
            "sq8": sq,
            "spb": sp,
            "mm8": np.concatenate(mm_blocks),
        })
    return in_maps


def _answer(outs):
    """outs: list of (stats [P,2], gram [P,256]) per core."""
    total = 0.0
    for stv, gv in outs:
        stv = stv.astype(np.float64)
        gv = gv.astype(np.float64)
        lnm = stv[:, 0:2].sum()
        trm = np.trace(gv[:, 0:P])
        trv = np.trace(gv[:, P:2 * P])
        total += 0.5 * (trm + trv) - lnm
    total -= 0.5 * (B * L * N * D)
    return np.array(total / (B * L), dtype=np.float32)


def _run(inputs, trace=False):
    nc = _get_nc()
    in_maps = _pack(inputs)
    res = None
    for attempt in range(3):
        try:
            res = run_bass_kernel_spmd(nc, in_maps, list(range(NCORES)),
                                       trace=trace)
            break
        except Exception:
            if attempt == 2:
                raise
            import time as _time
            _time.sleep(15)
    ans = _answer([(res.results[k]["stats"], res.results[k]["gram"])
                   for k in range(NCORES)])
    return ans, res


def kernel(prior_mu, prior_sigma, post_mu, post_sigma):
    inputs = {
        "prior_mu": np.asarray(prior_mu, dtype=np.float32),
        "prior_sigma": np.asarray(prior_sigma, dtype=np.float32),
        "post_mu": np.asarray(post_mu, dtype=np.float32),
        "post_sigma": np.asarray(post_sigma, dtype=np.float32),
    }
    ans, _ = _run(inputs, trace=False)
    return ans


# revision 4
# speedup vs baseline: 1.0407x; 1.0407x over previous
"""KL(N(prior_mu, diag(prior_sigma^2)) || N(post_mu, diag(post_sigma^2))) mean loss.

Data-parallel over batch dim B=32 across 8 NeuronCores (4 batches/core).
Host casts prior_sigma to bf16 and post_sigma/prior_mu/post_mu to fp8-e3m4
(5 B/elem -> 40 KiB/partition/core); the 2e-2 rel-err budget absorbs the
~0.1% quantization bias (measured rel err ~3e-3).

Math per element (sp, sq, mp, mq), with m = sp/sq and v = (mq-mp)/sq:
  kl = 0.5*m^2 + 0.5*v^2 - 0.5 - ln m
  Sum kl = 0.5*(Sum m^2 + Sum v^2) - N/2 - Sum ln m

Engine split per tile [128, w]:
  ACT : rcp = Reciprocal(sq) (PWP reciprocal table; the bass wrapper blocks
        Af.Reciprocal so the InstActivation is emitted directly -- one ACT
        pass instead of the Ln+Exp pair), two sq-stream DMAs, and the final
        root-Ln passes (one table switch, preloads hidden under DMA waits).
  DVE : m = sp*rcp, v = d*rcp (bf16 2x rate); tree levels l2..l4 for the
        previous tile; psum drains at the end.
  Pool: d = mq - mp (fp8 in, bf16 out); tree level l1; four sq-stream
        SWDGE DMAs.
  PE  : Sum m^2 and Sum v^2 as gram matrices G += X_chunk^T X_chunk
        accumulated over [128,128] chunks in two psum banks; the host reads
        the traces. This removes every square/STT pass from DVE.
  SP  : sp- and mm-stream DMAs, the output DMA.

Sum ln m comes from a bf16 pairwise product tree over m (groups of 16;
m in [1/3, 3] so group products stay in bf16 range) and ACT Ln+accum over
the [128, 512] tree roots in two chunks so most of it overlaps the last
tile. DMA is split across the SP, ACT and Pool DGE queues, which transfer
concurrently. Tiles are small at both ends ([512, 1536, 2048, 2048, 1536,
512]) to prime the pipeline quickly and keep the drain chain short.

Raw Bass (no Tile): standalone wait_ge synchronization with a schedule
prepass assigning per-engine ordinals, per-slot DMA semaphores, 3 DMA
slots, parity (2-slot) intermediate buffers.
"""

import sys
from contextlib import ExitStack

sys.path.insert(0, "/opt/trn_rl_repo")

import numpy as np
import ml_dtypes

import concourse.bass as bass
from concourse import mybir
from concourse.bass_utils import run_bass_kernel_spmd

B, L, N, D = 32, 128, 32, 64
NCORES = 8
BPC = B // NCORES
ELEMS = BPC * L * N * D          # 1_048_576 per tensor per core
P = 128
WIDTHS = [512, 1536, 2048, 2048, 1536, 512]
NT = len(WIDTHS)
assert sum(WIDTHS) * P == ELEMS
NSLOT = 3
GRP = 16                          # product-tree group size (4 levels)
SROOT = sum(w // GRP for w in WIDTHS)     # 512 tree-root columns
WMAX = max(WIDTHS)
ACT_SQ = (0, 1)                   # sq-stream tiles DMAd from the ACT queue
# output layout: [:, 0:2] root-Ln accums, [:, 2:130] G_m, [:, 130:258] G_v
OC = 2 + 2 * P

_CACHE = {}


def _build():
    dt = mybir.dt
    Af = mybir.ActivationFunctionType
    Op = mybir.AluOpType

    nc = bass.Bass()
    sq8 = nc.declare_dram_parameter("sq8", [ELEMS], dt.float8e3, isOutput=False)
    spb = nc.declare_dram_parameter("spb", [ELEMS], dt.bfloat16, isOutput=False)
    mm8 = nc.declare_dram_parameter("mm8", [2 * ELEMS], dt.float8e3,
                                    isOutput=False)
    out = nc.declare_dram_parameter("out", [P, OC], dt.float32, isOutput=True)

    off1 = [0]
    off2 = [0]
    for w in WIDTHS:
        off1.append(off1[-1] + P * w)
        off2.append(off2[-1] + P * 2 * w)

    def dram1(t, i):
        return t[off1[i]: off1[i + 1]].rearrange("(p f) -> p f", p=P)

    def dram2(t, i):
        return t[off2[i]: off2[i + 1]].rearrange("(p f) -> p f", p=P)

    soff = [0]
    for w in WIDTHS:
        soff.append(soff[-1] + w // GRP)

    # --- ordinal prepass (mirrors emission order exactly) ---
    # ACT sa: dummy_rcp=1; rcp_k=k+2; dummy_ln=NT+2; ln1=NT+3; ln2=NT+4
    rcpod = [k + 2 for k in range(NT)]
    ln2od = NT + 4
    # DVE sv: iteration k=0..NT: m_k, v_k (k<NT); l2,l3,l4 of k-1 (k>=1)
    mod_ = [0] * NT
    vod = [0] * NT
    l2od = [0] * NT
    l3od = [0] * NT
    l4od = [0] * NT
    nv = 0
    for k in range(NT + 1):
        if k < NT:
            nv += 1; mod_[k] = nv
            nv += 1; vod[k] = nv
        if k >= 1:
            m = k - 1
            nv += 1; l2od[m] = nv
            nv += 1; l3od[m] = nv
            nv += 1; l4od[m] = nv
    dgvod = nv + 2
    # Pool sg: d_k=2k+1, l1_k=2k+2
    dod = [2 * k + 1 for k in range(NT)]
    l1od = [2 * k + 2 for k in range(NT)]
    # PE se: gm_k=2k+1 (after last m-chunk), gv_k=2k+2
    gmod = [2 * k + 1 for k in range(NT)]
    gvod = [2 * k + 2 for k in range(NT)]
    pe_last = 2 * NT

    def ndma(i):
        return 16 * (i // NSLOT + 1)

    # sq tiles ride two queues; count waits per (queue, slot) semaphore
    sq_cnt = {}
    sq_wait = {}
    for k in range(NT):
        qn = "act" if k in ACT_SQ else "pool"
        key = (qn, k % NSLOT)
        sq_cnt[key] = sq_cnt.get(key, 0) + 16
        sq_wait[k] = (qn, k % NSLOT, sq_cnt[key])

    def raw_act(out_, in_, func, scale=1.0, accum_out=None):
        eng = nc.scalar
        ins = [eng.lower_ap(in_),
               mybir.ImmediateValue(dtype=dt.float32, value=0.0),
               mybir.ImmediateValue(dtype=dt.float32, value=scale),
               mybir.ImmediateValue(dtype=dt.float32, value=0.0)]
        outs = [eng.lower_ap(out_)]
        if accum_out is not None:
            outs.append(eng.lower_ap(accum_out))
        return eng.add_instruction(mybir.InstActivation(
            name=nc.get_next_instruction_name(), func=func, ins=ins, outs=outs))

    with ExitStack() as ctx:
        en = ctx.enter_context
        sq_b = [en(nc.sbuf_tensor(f"sq{i}", [P, WMAX], dt.float8e3))
                for i in range(NSLOT)]
        sp_b = [en(nc.sbuf_tensor(f"sp{i}", [P, WMAX], dt.bfloat16))
                for i in range(NSLOT)]
        mm_b = [en(nc.sbuf_tensor(f"mm{i}", [P, 2 * WMAX], dt.float8e3))
                for i in range(NSLOT)]
        rcp_b = [en(nc.sbuf_tensor(f"rcp{i}", [P, WMAX], dt.bfloat16))
                 for i in range(2)]
        m_b = [en(nc.sbuf_tensor(f"m{i}", [P, WMAX], dt.bfloat16))
               for i in range(2)]
        v_b = [en(nc.sbuf_tensor(f"v{i}", [P, WMAX], dt.bfloat16))
               for i in range(2)]
        d_b = [en(nc.sbuf_tensor(f"d{i}", [P, WMAX], dt.bfloat16))
               for i in range(2)]
        l1_b = [en(nc.sbuf_tensor(f"l1{i}", [P, WMAX // 2], dt.bfloat16))
                for i in range(2)]
        l2_b = [en(nc.sbuf_tensor(f"l2{i}", [P, WMAX // 4], dt.bfloat16))
                for i in range(2)]
        l3_b = [en(nc.sbuf_tensor(f"l3{i}", [P, WMAX // 8], dt.bfloat16))
                for i in range(2)]
        stash = en(nc.sbuf_tensor("stash", [P, SROOT], dt.bfloat16))
        lnout = en(nc.sbuf_tensor("lnout", [P, SROOT], dt.bfloat16))
        scr = en(nc.sbuf_tensor("scr", [P, 2], dt.bfloat16))
        gsb = en(nc.sbuf_tensor("gsb", [P, OC], dt.float32))
        Gm = en(nc.psum_tensor("Gm", [P, P], dt.float32))
        Gv = en(nc.psum_tensor("Gv", [P, P], dt.float32))

        dsq = [en(nc.semaphore(f"dsq{i}")) for i in range(NSLOT)]
        dsqp = [en(nc.semaphore(f"dsqp{i}")) for i in range(NSLOT)]
        dsp = [en(nc.semaphore(f"dsp{i}")) for i in range(NSLOT)]
        dmm = [en(nc.semaphore(f"dmm{i}")) for i in range(NSLOT)]
        sa = en(nc.semaphore("sa"))
        sv = en(nc.semaphore("sv"))
        sg = en(nc.semaphore("sg"))
        se = en(nc.semaphore("se"))
        do = en(nc.semaphore("do"))

        block = en(nc.Block())

        @block.sync
        def _(sync):
            for k in range(NT):
                s, w = k % NSLOT, WIDTHS[k]
                if k >= NSLOT:
                    sync.wait_ge(sv, mod_[k - NSLOT])   # sp slot WAR
                sync.dma_start(sp_b[s][:, 0:w], dram1(spb, k)).then_inc(dsp[s], 16)
                if k >= NSLOT:
                    sync.wait_ge(sg, dod[k - NSLOT])    # mm slot WAR
                sync.dma_start(mm_b[s][:, 0:2 * w], dram2(mm8, k)).then_inc(dmm[s], 16)
            sync.wait_ge(sa, ln2od)
            sync.wait_ge(sv, dgvod)
            sync.dma_start(out[:, :], gsb[:, :]).then_inc(do, 16)
            sync.wait_ge(do, 16)

        @block.scalar
        def _(scalar):
            ones = nc.const_aps.tensor(1.0, (P, 1), dt.float32)
            # first sq DMA ahead of the table preload so they overlap
            scalar.dma_start(sq_b[0][:, 0:WIDTHS[0]],
                             dram1(sq8, 0)).then_inc(dsq[0], 16)
            raw_act(scr[:, 0:1], ones, Af.Reciprocal).then_inc(sa, 1)
            scalar.dma_start(sq_b[1][:, 0:WIDTHS[1]],
                             dram1(sq8, 1)).then_inc(dsq[1], 16)
            for k in range(NT):
                s, w = k % NSLOT, WIDTHS[k]
                qn, ss, cnt = sq_wait[k]
                scalar.wait_ge(dsq[ss] if qn == "act" else dsqp[ss], cnt)
                if k >= 2:
                    scalar.wait_ge(sv, vod[k - 2])      # rcp WAR vs DVE v
                raw_act(rcp_b[k % 2][:, 0:w], sq_b[s][:, 0:w],
                        Af.Reciprocal).then_inc(sa, 1)
            # switch to the natural_log table while the tree drains
            nc.scalar.activation(scr[:, 1:2], ones, Af.Ln).then_inc(sa, 1)
            # root-Ln part 1: tiles 0..NT-2
            scalar.wait_ge(sv, l4od[NT - 2])
            nc.scalar.activation(
                lnout[:, 0: soff[NT - 1]], stash[:, 0: soff[NT - 1]], Af.Ln,
                accum_out=gsb[:, 0:1]).then_inc(sa, 1)
            # part 2: last tile's roots
            scalar.wait_ge(sv, l4od[NT - 1])
            nc.scalar.activation(
                lnout[:, soff[NT - 1]: SROOT], stash[:, soff[NT - 1]: SROOT],
                Af.Ln, accum_out=gsb[:, 1:2]).then_inc(sa, 1)

        @block.vector
        def _(vector):
            for k in range(NT + 1):
                if k < NT:
                    s, j, w = k % NSLOT, k % 2, WIDTHS[k]
                    vector.wait_ge(sa, rcpod[k])        # rcp RAW
                    vector.wait_ge(dsp[s], ndma(k))
                    if k >= 2:
                        vector.wait_ge(sg, l1od[k - 2])  # m WAR vs Pool l1
                        vector.wait_ge(se, gmod[k - 2])  # m WAR vs PE
                    nc.vector.tensor_tensor(
                        m_b[j][:, 0:w], sp_b[s][:, 0:w], rcp_b[j][:, 0:w],
                        op=Op.mult).then_inc(sv, 1)
                    vector.wait_ge(sg, dod[k])           # d RAW
                    if k >= 2:
                        vector.wait_ge(se, gvod[k - 2])  # v WAR vs PE
                    nc.vector.tensor_tensor(
                        v_b[j][:, 0:w], d_b[j][:, 0:w], rcp_b[j][:, 0:w],
                        op=Op.mult).then_inc(sv, 1)
                if k >= 1:
                    m = k - 1
                    jm, wm = m % 2, WIDTHS[m]
                    vector.wait_ge(sg, l1od[m])          # l1 RAW (Pool)
                    if m >= 2:
                        vector.wait_ge(sv, l3od[m - 2])  # l2 WAR (self)
                    nc.vector.tensor_tensor(
                        l2_b[jm][:, 0:wm // 4], l1_b[jm][:, 0:wm // 4],
                        l1_b[jm][:, wm // 4:wm // 2], op=Op.mult).then_inc(sv, 1)
                    vector.wait_ge(sv, l2od[m])          # l2 RAW (self)
                    if m >= 2:
                        vector.wait_ge(sv, l4od[m - 2])  # l3 WAR (self)
                    nc.vector.tensor_tensor(
                        l3_b[jm][:, 0:wm // 8], l2_b[jm][:, 0:wm // 8],
                        l2_b[jm][:, wm // 8:wm // 4], op=Op.mult).then_inc(sv, 1)
                    vector.wait_ge(sv, l3od[m])          # l3 RAW (self)
                    nc.vector.tensor_tensor(
                        stash[:, soff[m]: soff[m + 1]], l3_b[jm][:, 0:wm // 16],
                        l3_b[jm][:, wm // 16:wm // 8], op=Op.mult).then_inc(sv, 1)
            # psum drains
            vector.wait_ge(se, pe_last)
            nc.vector.tensor_copy(gsb[:, 2:2 + P], Gm[:, :]).then_inc(sv, 1)
            nc.vector.tensor_copy(gsb[:, 2 + P:OC], Gv[:, :]).then_inc(sv, 1)

        @block.gpsimd
        def _(gpsimd):
            for k in range(NT):
                s, j, w = k % NSLOT, k % 2, WIDTHS[k]
                kq = k + 2
                if 2 <= kq < NT and kq not in ACT_SQ:
                    sq_s = kq % NSLOT
                    if kq >= NSLOT:
                        gpsimd.wait_ge(sa, rcpod[kq - NSLOT])  # sq slot WAR
                    gpsimd.dma_start(sq_b[sq_s][:, 0:WIDTHS[kq]],
                                     dram1(sq8, kq)).then_inc(
                                         dsqp[sq_wait[kq][1]],
                                         16)
                gpsimd.wait_ge(dmm[s], ndma(k))
                if k >= 2:
                    gpsimd.wait_ge(sv, vod[k - 2])      # d WAR vs DVE v
                nc.gpsimd.tensor_tensor(
                    d_b[j][:, 0:w], mm_b[s][:, w:2 * w], mm_b[s][:, 0:w],
                    op=Op.subtract).then_inc(sg, 1)
                gpsimd.wait_ge(sv, mod_[k])             # m RAW
                if k >= 2:
                    gpsimd.wait_ge(sv, l2od[k - 2])     # l1 WAR vs DVE l2
                nc.gpsimd.tensor_tensor(
                    l1_b[j][:, 0:w // 2], m_b[j][:, 0:w // 2],
                    m_b[j][:, w // 2:w], op=Op.mult).then_inc(sg, 1)

        @block.tensor
        def _(tensor):
            for k in range(NT):
                j, w = k % 2, WIDTHS[k]
                nch = w // P
                tensor.wait_ge(sv, mod_[k])
                for c in range(nch):
                    mm_ = nc.tensor.matmul(
                        Gm[:, :], m_b[j][:, c * P:(c + 1) * P],
                        m_b[j][:, c * P:(c + 1) * P],
                        start=(k == 0 and c == 0),
                        stop=(k == NT - 1 and c == nch - 1),
                        skip_group_check=True)
                    if c == nch - 1:
                        mm_.then_inc(se, 1)
                tensor.wait_ge(sv, vod[k])
                for c in range(nch):
                    mm_ = nc.tensor.matmul(
                        Gv[:, :], v_b[j][:, c * P:(c + 1) * P],
                        v_b[j][:, c * P:(c + 1) * P],
                        start=(k == 0 and c == 0),
                        stop=(k == NT - 1 and c == nch - 1),
                        skip_group_check=True)
                    if c == nch - 1:
                        mm_.then_inc(se, 1)

    return nc


def _get_nc():
    if "nc" not in _CACHE:
        _CACHE["nc"] = _build()
    return _CACHE["nc"]


def _pack(inputs):
    """Per-core packed streams, tile-blocked to match the kernel's DRAM APs:
    sq8 = post_sigma fp8-e3m4; spb = prior_sigma bf16;
    mm8 = per tile [prior_mu | post_mu] fp8-e3m4 (so d = cols w:2w - 0:w)."""
    e3 = ml_dtypes.float8_e3m4
    bf = ml_dtypes.bfloat16
    in_maps = []
    for k in range(NCORES):
        sl = slice(k * BPC, (k + 1) * BPC)
        sq = np.ascontiguousarray(inputs["post_sigma"][sl]).reshape(-1).astype(e3)
        sp = np.ascontiguousarray(inputs["prior_sigma"][sl]).reshape(-1).astype(bf)
        mp = np.ascontiguousarray(inputs["prior_mu"][sl]).reshape(-1).astype(e3)
        mq = np.ascontiguousarray(inputs["post_mu"][sl]).reshape(-1).astype(e3)
        mm_blocks = []
        pos = 0
        for w in WIDTHS:
            n = P * w
            a = mp[pos:pos + n].reshape(P, w)
            b = mq[pos:pos + n].reshape(P, w)
            mm_blocks.append(np.concatenate([a, b], axis=1).ravel())
            pos += n
        in_maps.append({
            "sq8": sq,
            "spb": sp,
            "mm8": np.concatenate(mm_blocks),
        })
    return in_maps


def _answer(outs):
    """outs: list of out [P, 258] arrays per core."""
    total = 0.0
    for ov in outs:
        ov = ov.astype(np.float64)
        lnm = ov[:, 0:2].sum()
        trm = np.trace(ov[:, 2:2 + P])
        trv = np.trace(ov[:, 2 + P:OC])
        total += 0.5 * (trm + trv) - lnm
    total -= 0.5 * (B * L * N * D)
    return np.array(total / (B * L), dtype=np.float32)


def _run(inputs, trace=False):
    nc = _get_nc()
    in_maps = _pack(inputs)
    res = None
    for attempt in range(3):
        try:
            res = run_bass_kernel_spmd(nc, in_maps, list(range(NCORES)),
                                       trace=trace)
            break
        except Exception:
            if attempt == 2:
                raise
            import time as _time
            _time.sleep(15)
    ans = _answer([res.results[k]["out"] for k in range(NCORES)])
    return ans, res


def kernel(prior_mu, prior_sigma, post_mu, post_sigma):
    inputs = {
        "prior_mu": np.asarray(prior_mu, dtype=np.float32),
        "prior_sigma": np.asarray(prior_sigma, dtype=np.float32),
        "post_mu": np.asarray(post_mu, dtype=np.float32),
        "post_sigma": np.asarray(post_sigma, dtype=np.float32),
    }
    ans, _ = _run(inputs, trace=False)
    return ans
